# revision 24
# baseline (speedup 1.0000x reference)
"""Trainium2 Bass kernel for nn_MultiHeadAttention_68410239091266.

Contract: kernel(**inputs) takes the FULL unsharded inputs of the reference
(Q, K, V, Wq, bq, Wk, bk, Wv, bv, Wo, bo) and returns the full output tuple
(out [2,2048,768] f32, weights [2,12,2048,2048] f32).

Sharding: batch*head parallel across 8 cores. Core c handles batch c//4,
heads (c%4)*3 .. (c%4)*3+2 (12 heads / 4 cores per batch = 3 heads/core).
Host pre-transposes per-batch activations (Q^T/K^T/V^T) and per-core weight
slices so the device kernel needs no large on-chip transposes.

Device pipeline per head:
  P : q^T,k^T [64,S] via fp32r matmuls (scale 1/sqrt(dh) folded into Wq on
      host); v [S,64] in bf16.
  S1: scores[q,k] tiles -> exp on ACT with free-dim accum => rowsums Z;
      reciprocal r=1/Z; normalize (tensor_scalar, per-partition) -> DMA the
      402MB weights output (the memory-roofline stream).
  -lnZ row: Ln(r) -> PE transpose -> DRAM roundtrip -> row 64 of qT_aug.
  S2: scoresT'[k,q] = k^T.T@q^T with K=65 (row 64: ones x -lnZ_q) -> exp =>
      NORMALIZED probsT in bf16 (softmax fold via exp(s - lnZ)).
  PV: attnT[d,q] = sum_k v[k,d] probsT[k,q]  (bf16, PSUM accumulation).
  O : out[s,:] += sum_h attnT_h.T @ Wo_h^T   (bf16), one PSUM accumulation.
Host: sums the 4 per-batch partial outs, adds (bv @ Wo^T + bo), and
reassembles the per-core weight blocks.
"""

import numpy as np
import ml_dtypes

D = 768          # d_model
NH = 12          # num heads
DH = 64          # head depth
B = 2
S = 2048
HPC = 3          # heads per core
NCORES = 8
SCALE = 1.0 / np.sqrt(DH)

_NC_CACHE = {}


def build_nc(s=S):
    """Build the per-core Bass program (same program for all 8 cores)."""
    from contextlib import ExitStack

    import concourse.tile as tile
    from concourse import bacc, mybir
    from concourse.bass import MemorySpace
    from concourse.masks import make_identity

    f32 = mybir.dt.float32
    f32r = mybir.dt.float32r
    bf16 = mybir.dt.bfloat16
    EXP = mybir.ActivationFunctionType.Exp
    LN = mybir.ActivationFunctionType.Ln

    CCH = D // 128           # 6 contraction chunks for projections
    NQT = s // 128           # query/key 128-row tiles
    NCH = s // 512           # 512-wide chunks
    NSH = s // 1024          # 1024-wide halves

    nc = bacc.Bacc()

    xqT = nc.dram_tensor("xqT", [D, s], f32r, kind="ExternalInput")
    xkT = nc.dram_tensor("xkT", [D, s], f32r, kind="ExternalInput")
    xvT = nc.dram_tensor("xvT", [D, s], bf16, kind="ExternalInput")
    wqT = nc.dram_tensor("wqT", [D, HPC * DH], f32r, kind="ExternalInput")
    wkT = nc.dram_tensor("wkT", [D, HPC * DH], f32r, kind="ExternalInput")
    wvT = nc.dram_tensor("wvT", [D, HPC * DH], bf16, kind="ExternalInput")
    woT = nc.dram_tensor("woT", [DH, HPC, D], bf16, kind="ExternalInput")
    bqd = nc.dram_tensor("bq", [DH, HPC], f32, kind="ExternalInput")
    bkd = nc.dram_tensor("bk", [DH, HPC], f32, kind="ExternalInput")
    w_out = nc.dram_tensor("w_out", [HPC, s, s], f32, kind="ExternalOutput")
    o_out = nc.dram_tensor("o_out", [s, D], f32, kind="ExternalOutput")
    lnz_dram = [nc.dram_tensor(f"lnz_scratch{h}", [1, s], f32r) for h in range(HPC)]
    onesd = nc.dram_tensor("ones", [1, s], f32r, kind="ExternalInput")

    with tile.TileContext(nc) as tc, ExitStack() as ctx:
        consts = ctx.enter_context(tc.tile_pool(name="consts", bufs=1))
        xpool = ctx.enter_context(tc.tile_pool(name="xpool", bufs=3))
        qk = ctx.enter_context(tc.tile_pool(name="qk", bufs=1))
        epool = ctx.enter_context(tc.tile_pool(name="epool", bufs=3))
        etpool = ctx.enter_context(tc.tile_pool(name="etpool", bufs=1))
        opool = ctx.enter_context(tc.tile_pool(name="opool", bufs=3))
        stat = ctx.enter_context(tc.tile_pool(name="stat", bufs=2))
        psum = ctx.enter_context(
            tc.tile_pool(name="psum", bufs=2, space=MemorySpace.PSUM)
        )

        # ---- constants ----
        wq_sb = consts.tile([128, CCH, HPC * DH], f32r, tag="wq")
        nc.sync.dma_start(wq_sb[:], wqT.rearrange("(n p) m -> p n m", p=128))
        wk_sb = consts.tile([128, CCH, HPC * DH], f32r, tag="wk")
        nc.sync.dma_start(wk_sb[:], wkT.rearrange("(n p) m -> p n m", p=128))
        wv_sb = consts.tile([128, CCH, HPC * DH], bf16, tag="wv")
        nc.sync.dma_start(wv_sb[:], wvT.rearrange("(n p) m -> p n m", p=128))
        wo_sb = consts.tile([DH, HPC, D], bf16, tag="wo")
        nc.sync.dma_start(wo_sb[:], woT[:])
        bq_sb = consts.tile([DH, HPC], f32, tag="bq")
        nc.sync.dma_start(bq_sb[:], bqd[:])
        bk_sb = consts.tile([DH, HPC], f32, tag="bk")
        nc.sync.dma_start(bk_sb[:], bkd[:])
        ident = consts.tile([128, 128], f32, tag="ident")
        make_identity(nc, ident[:])

        # ---- persistent per-head tiles ----
        # qT_aug/kT_aug: rows 0..63 = projection output; row 64: kT -> ones,
        # qT -> -ln(Z) (written after S1).
        qT_aug = [
            qk.tile([65, s], f32r, tag=f"qT{h}", name=f"qT{h}") for h in range(HPC)
        ]
        kT_aug = [
            qk.tile([65, s], f32r, tag=f"kT{h}", name=f"kT{h}") for h in range(HPC)
        ]
        v_sb = [
            qk.tile([128, NQT, DH], bf16, tag=f"v{h}", name=f"v{h}")
            for h in range(HPC)
        ]
        attnT = [
            qk.tile([DH, s], bf16, tag=f"attnT{h}", name=f"attnT{h}")
            for h in range(HPC)
        ]
        for h in range(HPC):
            nc.sync.dma_start(kT_aug[h][64:65, :], onesd[:])

        # ---- phase P: q/k projections (fp32r, output transposed) ----
        for xdram, wsb, dsts, bsb in (
            (xqT, wq_sb, qT_aug, bq_sb),
            (xkT, wk_sb, kT_aug, bk_sb),
        ):
            for sh in range(NSH):
                pss = [
                    psum.tile([64, HPC, 512], f32, tag="big", name=f"psp{i}")
                    for i in range(2)
                ]
                for c in range(CCH):
                    xt = xpool.tile([128, 1024], f32r, tag="x")
                    nc.sync.dma_start(
                        xt[:],
                        xdram[c * 128 : (c + 1) * 128, sh * 1024 : (sh + 1) * 1024],
                    )
                    for qtr in range(2):
                        rhs = xt[:, qtr * 512 : (qtr + 1) * 512]
                        for h in range(HPC):
                            nc.tensor.matmul(
                                pss[qtr][:, h, :],
                                lhsT=wsb[:, c, h * DH : (h + 1) * DH],
                                rhs=rhs,
                                start=(c == 0),
                                stop=(c == CCH - 1),
                            )
                for qtr in range(2):
                    s0 = sh * 1024 + qtr * 512
                    for h in range(HPC):
                        nc.vector.tensor_scalar_add(
                            dsts[h][0:64, s0 : s0 + 512],
                            pss[qtr][:, h, :],
                            bsb[:, h : h + 1],
                        )

        # ---- phase P: v projection (bf16, natural [s, dh] layout) ----
        for sh in range(NSH):
            psv = [
                psum.tile([128, 4, 512], f32, tag="big", name=f"psv{i}")
                for i in range(2)
            ]
            for c in range(CCH):
                xt = xpool.tile([128, 1024], bf16, tag="x")
                nc.sync.dma_start(
                    xt[:], xvT[c * 128 : (c + 1) * 128, sh * 1024 : (sh + 1) * 1024]
                )
                for sb in range(8):
                    nc.tensor.matmul(
                        psv[sb // 4][:, sb % 4, 0 : HPC * DH],
                        lhsT=xt[:, sb * 128 : (sb + 1) * 128],
                        rhs=wv_sb[:, c, :],
                        start=(c == 0),
                        stop=(c == CCH - 1),
                    )
            for sb in range(8):
                for h in range(HPC):
                    nc.vector.tensor_copy(
                        v_sb[h][:, sh * 8 + sb, :],
                        psv[sb // 4][:, sb % 4, h * DH : (h + 1) * DH],
                    )

        # ---- per-head attention ----
        for h in range(HPC):
            qT = qT_aug[h]
            kT = kT_aug[h]
            z_all = stat.tile([128, NQT], f32, tag="z")
            r_all = stat.tile([128, NQT], f32, tag="r")

            # S1: scores [q,k], exp+rowsum, normalize, store weights
            for qt in range(NQT):
                ps = psum.tile([128, s], f32, tag="big")
                lhsT = qT[0:64, qt * 128 : (qt + 1) * 128]
                for kc in range(NCH):
                    nc.tensor.matmul(
                        ps[:, kc * 512 : (kc + 1) * 512],
                        lhsT=lhsT,
                        rhs=kT[0:64, kc * 512 : (kc + 1) * 512],
                        start=True,
                        stop=True,
                    )
                et = epool.tile([128, s], f32, tag="e")
                nc.scalar.activation(
                    et[:], ps[:], EXP, accum_out=z_all[:, qt : qt + 1]
                )
                nc.vector.reciprocal(r_all[:, qt : qt + 1], z_all[:, qt : qt + 1])
                nc.vector.tensor_scalar_mul(et[:], et[:], r_all[:, qt : qt + 1])
                nc.sync.dma_start(w_out[h, qt * 128 : (qt + 1) * 128, :], et[:])

            # -lnZ: Ln(r) -> transpose -> DRAM roundtrip -> qT_aug row 64
            nl = stat.tile([128, NQT], f32, tag="nl")
            nc.scalar.activation(nl[:], r_all[:], LN)
            pst = psum.tile([NQT, 128], f32, tag="big")
            nc.tensor.transpose(pst[:], nl[:], ident[:])
            stg = stat.tile([NQT, 128], f32r, tag="stg")
            nc.vector.tensor_copy(stg[:], pst[:])
            nc.sync.dma_start(
                lnz_dram[h].rearrange("o (a b) -> (o a) b", a=NQT), stg[:]
            )
            nc.sync.dma_start(qT[64:65, :], lnz_dram[h][:])

            # S2: scoresT' = kT_aug.T @ qT_aug (K=65 folds -lnZ), exp -> bf16
            et_full = etpool.tile([128, NQT, s], bf16, tag="eT")
            for kt in range(NQT):
                ps = psum.tile([128, s], f32, tag="big")
                lhsT = kT[0:65, kt * 128 : (kt + 1) * 128]
                for qc in range(NCH):
                    nc.tensor.matmul(
                        ps[:, qc * 512 : (qc + 1) * 512],
                        lhsT=lhsT,
                        rhs=qT[0:65, qc * 512 : (qc + 1) * 512],
                        start=True,
                        stop=True,
                    )
                nc.scalar.activation(et_full[:, kt, :], ps[:], EXP)

            # PV: attnT[d, q] = sum_k v[k, d] * probsT[k, q]
            pa = psum.tile([DH, s], f32, tag="big")
            for kt in range(NQT):
                for qc in range(NCH):
                    nc.tensor.matmul(
                        pa[:, qc * 512 : (qc + 1) * 512],
                        lhsT=v_sb[h][:, kt, :],
                        rhs=et_full[:, kt, qc * 512 : (qc + 1) * 512],
                        start=(kt == 0),
                        stop=(kt == NQT - 1),
                    )
            nc.vector.tensor_copy(attnT[h][:], pa[:])

        # ---- phase O: out[s,:] = sum_h attnT_h.T @ Wo_h^T ----
        for sb in range(NQT):
            po = psum.tile([128, D], f32, tag="big")
            for h in range(HPC):
                lhsT = attnT[h][:, sb * 128 : (sb + 1) * 128]
                nc.tensor.matmul(
                    po[:, 0:512],
                    lhsT=lhsT,
                    rhs=wo_sb[:, h, 0:512],
                    start=(h == 0),
                    stop=(h == HPC - 1),
                )
                nc.tensor.matmul(
                    po[:, 512:768],
                    lhsT=lhsT,
                    rhs=wo_sb[:, h, 512:768],
                    start=(h == 0),
                    stop=(h == HPC - 1),
                )
            ot = opool.tile([128, D], f32, tag="o")
            nc.vector.tensor_copy(ot[:], po[:])
            nc.sync.dma_start(o_out[sb * 128 : (sb + 1) * 128, :], ot[:])

    nc.finalize()
    return nc


def _get_nc():
    if "nc" not in _NC_CACHE:
        _NC_CACHE["nc"] = build_nc()
    return _NC_CACHE["nc"]


def make_in_maps(Q, K, V, Wq, bq, Wk, bk, Wv, bv, Wo, bo, s=S):
    bf = ml_dtypes.bfloat16
    QT = [np.ascontiguousarray(Q[b].T) for b in range(B)]
    KT = [np.ascontiguousarray(K[b].T) for b in range(B)]
    VT = [np.ascontiguousarray(V[b].T).astype(bf) for b in range(B)]
    in_maps = []
    for core in range(NCORES):
        b = core // (NCORES // B)
        h0 = (core % (NCORES // B)) * HPC
        sl = slice(h0 * DH, (h0 + HPC) * DH)
        in_maps.append(
            {
                "xqT": QT[b],
                "xkT": KT[b],
                "xvT": VT[b],
                "wqT": np.ascontiguousarray((Wq[sl, :] * SCALE).T),
                "wkT": np.ascontiguousarray(Wk[sl, :].T),
                "wvT": np.ascontiguousarray(Wv[sl, :].T).astype(bf),
                "woT": np.ascontiguousarray(
                    Wo[:, sl].T.reshape(HPC, DH, D).transpose(1, 0, 2)
                ).astype(bf),
                "bq": np.ascontiguousarray((bq[sl] * SCALE).reshape(HPC, DH).T),
                "bk": np.ascontiguousarray(bk[sl].reshape(HPC, DH).T),
                "ones": np.ones((1, s), np.float32),
            }
        )
    return in_maps


def kernel(**inputs):
    Q = np.asarray(inputs["Q"], np.float32)
    K = np.asarray(inputs["K"], np.float32)
    V = np.asarray(inputs["V"], np.float32)
    Wq = np.asarray(inputs["Wq"], np.float32)
    bq = np.asarray(inputs["bq"], np.float32)
    Wk = np.asarray(inputs["Wk"], np.float32)
    bk = np.asarray(inputs["bk"], np.float32)
    Wv = np.asarray(inputs["Wv"], np.float32)
    bv = np.asarray(inputs["bv"], np.float32)
    Wo = np.asarray(inputs["Wo"], np.float32)
    bo = np.asarray(inputs["bo"], np.float32)

    import os

    from concourse.bass_utils import run_bass_kernel_spmd

    nc = _get_nc()
    in_maps = make_in_maps(Q, K, V, Wq, bq, Wk, bk, Wv, bv, Wo, bo)
    res = run_bass_kernel_spmd(
        nc, in_maps, list(range(NCORES)), trace=bool(os.environ.get("MHA_TRACE"))
    )
    _NC_CACHE["last_res"] = res

    weights = np.empty((B, NH, S, S), np.float32)
    out = np.zeros((B, S, D), np.float32)
    for core in range(NCORES):
        b = core // (NCORES // B)
        h0 = (core % (NCORES // B)) * HPC
        weights[b, h0 : h0 + HPC] = res.results[core]["w_out"]
        out[b] += res.results[core]["o_out"]
    out += (bv @ Wo.T + bo)[None, None, :]
    return (out, weights)


# revision 25
# speedup vs baseline: 1.1305x; 1.1305x over previous
"""Trainium2 Bass kernel for nn_MultiHeadAttention_68410239091266.

Contract: kernel(**inputs) takes the FULL unsharded inputs of the reference
(Q, K, V, Wq, bq, Wk, bk, Wv, bv, Wo, bo) and returns the full output tuple
(out [2,2048,768] f32, weights [2,12,2048,2048] f32).

Sharding: batch*head parallel across 8 cores. Core c handles batch c//4,
heads (c%4)*3 .. (c%4)*3+2 (12 heads / 4 cores per batch = 3 heads/core).
Host pre-transposes per-batch activations (Q^T/K^T/V^T) and per-core weight
slices so the device kernel needs no large on-chip transposes.

Device pipeline per head:
  P : q^T,k^T [64,S] via fp32r matmuls (scale 1/sqrt(dh) folded into Wq on
      host); v [S,64] in bf16.
  S1: scores[q,k] tiles -> exp on ACT with free-dim accum => rowsums Z;
      reciprocal r=1/Z; normalize (tensor_scalar, per-partition) -> DMA the
      402MB weights output (the memory-roofline stream).
  -lnZ row: Ln(r) -> PE transpose -> DRAM roundtrip -> row 64 of qT_aug.
  S2: scoresT'[k,q] = k^T.T@q^T with K=65 (row 64: ones x -lnZ_q) -> exp =>
      NORMALIZED probsT in bf16 (softmax fold via exp(s - lnZ)).
  PV: attnT[d,q] = sum_k v[k,d] probsT[k,q]  (bf16, PSUM accumulation).
  O : out[s,:] += sum_h attnT_h.T @ Wo_h^T   (bf16), one PSUM accumulation.
Host: sums the 4 per-batch partial outs, adds (bv @ Wo^T + bo), and
reassembles the per-core weight blocks.
"""

import numpy as np
import ml_dtypes

D = 768          # d_model
NH = 12          # num heads
DH = 64          # head depth
B = 2
S = 2048
HPC = 3          # heads per core
NCORES = 8
SCALE = 1.0 / np.sqrt(DH)

_NC_CACHE = {}


def build_nc(s=S):
    """Build the per-core Bass program (same program for all 8 cores)."""
    from contextlib import ExitStack

    import concourse.tile as tile
    from concourse import bacc, mybir
    from concourse.bass import MemorySpace
    from concourse.masks import make_identity

    f32 = mybir.dt.float32
    f32r = mybir.dt.float32r
    bf16 = mybir.dt.bfloat16
    EXP = mybir.ActivationFunctionType.Exp
    LN = mybir.ActivationFunctionType.Ln

    CCH = D // 128           # 6 contraction chunks for projections
    NQT = s // 128           # query/key 128-row tiles
    NCH = s // 512           # 512-wide chunks
    NSH = s // 1024          # 1024-wide halves

    nc = bacc.Bacc()

    xqT = nc.dram_tensor("xqT", [D, s], f32r, kind="ExternalInput")
    xkT = nc.dram_tensor("xkT", [D, s], f32r, kind="ExternalInput")
    xvT = nc.dram_tensor("xvT", [D, s], bf16, kind="ExternalInput")
    wqT = nc.dram_tensor("wqT", [D, HPC * DH], f32r, kind="ExternalInput")
    wkT = nc.dram_tensor("wkT", [D, HPC * DH], f32r, kind="ExternalInput")
    wvT = nc.dram_tensor("wvT", [D, HPC * DH], bf16, kind="ExternalInput")
    woT = nc.dram_tensor("woT", [DH, HPC, D], bf16, kind="ExternalInput")
    bqd = nc.dram_tensor("bq", [DH, HPC], f32, kind="ExternalInput")
    bkd = nc.dram_tensor("bk", [DH, HPC], f32, kind="ExternalInput")
    w_out = nc.dram_tensor("w_out", [HPC, s, s], f32, kind="ExternalOutput")
    o_out = nc.dram_tensor("o_out", [s, D], f32, kind="ExternalOutput")
    lnz_dram = [nc.dram_tensor(f"lnz_scratch{h}", [1, s], f32r) for h in range(HPC)]
    onesd = nc.dram_tensor("ones", [1, s], f32r, kind="ExternalInput")

    with tile.TileContext(nc) as tc, ExitStack() as ctx:
        consts = ctx.enter_context(tc.tile_pool(name="consts", bufs=1))
        xpool = ctx.enter_context(tc.tile_pool(name="xpool", bufs=3))
        qk = ctx.enter_context(tc.tile_pool(name="qk", bufs=1))
        epool = ctx.enter_context(tc.tile_pool(name="epool", bufs=3))
        etpool = ctx.enter_context(tc.tile_pool(name="etpool", bufs=1))
        opool = ctx.enter_context(tc.tile_pool(name="opool", bufs=3))
        stat = ctx.enter_context(tc.tile_pool(name="stat", bufs=2))
        psum = ctx.enter_context(
            tc.tile_pool(name="psum", bufs=2, space=MemorySpace.PSUM)
        )

        # ---- constants ----
        wq_sb = consts.tile([128, CCH, HPC * DH], f32r, tag="wq")
        nc.sync.dma_start(wq_sb[:], wqT.rearrange("(n p) m -> p n m", p=128))
        wk_sb = consts.tile([128, CCH, HPC * DH], f32r, tag="wk")
        nc.sync.dma_start(wk_sb[:], wkT.rearrange("(n p) m -> p n m", p=128))
        wv_sb = consts.tile([128, CCH, HPC * DH], bf16, tag="wv")
        nc.sync.dma_start(wv_sb[:], wvT.rearrange("(n p) m -> p n m", p=128))
        wo_sb = consts.tile([DH, HPC, D], bf16, tag="wo")
        nc.sync.dma_start(wo_sb[:], woT[:])
        bq_sb = consts.tile([DH, HPC], f32, tag="bq")
        nc.sync.dma_start(bq_sb[:], bqd[:])
        bk_sb = consts.tile([DH, HPC], f32, tag="bk")
        nc.sync.dma_start(bk_sb[:], bkd[:])
        ident = consts.tile([128, 128], f32, tag="ident")
        make_identity(nc, ident[:])

        # ---- persistent per-head tiles ----
        # qT_aug/kT_aug: rows 0..63 = projection output; row 64: kT -> ones,
        # qT -> -ln(Z) (written after S1).
        qT_aug = [
            qk.tile([65, s], f32r, tag=f"qT{h}", name=f"qT{h}") for h in range(HPC)
        ]
        kT_aug = [
            qk.tile([65, s], f32r, tag=f"kT{h}", name=f"kT{h}") for h in range(HPC)
        ]
        v_sb = [
            qk.tile([128, NQT, DH], bf16, tag=f"v{h}", name=f"v{h}")
            for h in range(HPC)
        ]
        attnT = [
            qk.tile([DH, s], bf16, tag=f"attnT{h}", name=f"attnT{h}")
            for h in range(HPC)
        ]
        for h in range(HPC):
            nc.sync.dma_start(kT_aug[h][64:65, :], onesd[:])

        # ---- phase P: q/k projections (fp32r, output transposed) ----
        for xdram, wsb, dsts, bsb in (
            (xqT, wq_sb, qT_aug, bq_sb),
            (xkT, wk_sb, kT_aug, bk_sb),
        ):
            for sh in range(NSH):
                pss = [
                    psum.tile([64, HPC, 512], f32, tag="big", name=f"psp{i}")
                    for i in range(2)
                ]
                for c in range(CCH):
                    xt = xpool.tile([128, 1024], f32r, tag="x")
                    nc.sync.dma_start(
                        xt[:],
                        xdram[c * 128 : (c + 1) * 128, sh * 1024 : (sh + 1) * 1024],
                    )
                    for qtr in range(2):
                        rhs = xt[:, qtr * 512 : (qtr + 1) * 512]
                        for h in range(HPC):
                            nc.tensor.matmul(
                                pss[qtr][:, h, :],
                                lhsT=wsb[:, c, h * DH : (h + 1) * DH],
                                rhs=rhs,
                                start=(c == 0),
                                stop=(c == CCH - 1),
                            )
                for qtr in range(2):
                    s0 = sh * 1024 + qtr * 512
                    for h in range(HPC):
                        nc.vector.tensor_scalar_add(
                            dsts[h][0:64, s0 : s0 + 512],
                            pss[qtr][:, h, :],
                            bsb[:, h : h + 1],
                        )

        # ---- phase P: v projection (bf16, natural [s, dh] layout) ----
        for sh in range(NSH):
            psv = [
                psum.tile([128, 4, 512], f32, tag="big", name=f"psv{i}")
                for i in range(2)
            ]
            for c in range(CCH):
                xt = xpool.tile([128, 1024], bf16, tag="x")
                nc.sync.dma_start(
                    xt[:], xvT[c * 128 : (c + 1) * 128, sh * 1024 : (sh + 1) * 1024]
                )
                for sb in range(8):
                    nc.tensor.matmul(
                        psv[sb // 4][:, sb % 4, 0 : HPC * DH],
                        lhsT=xt[:, sb * 128 : (sb + 1) * 128],
                        rhs=wv_sb[:, c, :],
                        start=(c == 0),
                        stop=(c == CCH - 1),
                    )
            for sb in range(8):
                for h in range(HPC):
                    nc.vector.tensor_copy(
                        v_sb[h][:, sh * 8 + sb, :],
                        psv[sb // 4][:, sb % 4, h * DH : (h + 1) * DH],
                    )

        # ---- per-head attention ----
        # S1(h+1) is interleaved tile-by-tile with S2a(h) so the Scalar
        # engine (exp) stays saturated and the weights-store stream drains
        # over a 2x wider window (S1 alone saturates HBM write bandwidth).
        z_all = [stat.tile([128, NQT], f32, tag=f"z{h}", name=f"z{h}") for h in range(HPC)]
        r_all = [stat.tile([128, NQT], f32, tag=f"r{h}", name=f"r{h}") for h in range(HPC)]
        et_fulls = [None] * HPC

        def s1_tile(h, qt):
            qT = qT_aug[h]
            kT = kT_aug[h]
            ps = psum.tile([128, s], f32, tag="big", name=f"s1_{h}_{qt}")
            lhsT = qT[0:64, qt * 128 : (qt + 1) * 128]
            for kc in range(NCH):
                nc.tensor.matmul(
                    ps[:, kc * 512 : (kc + 1) * 512],
                    lhsT=lhsT,
                    rhs=kT[0:64, kc * 512 : (kc + 1) * 512],
                    start=True,
                    stop=True,
                )
            et = epool.tile([128, s], f32, tag="e")
            nc.scalar.activation(
                et[:], ps[:], EXP, accum_out=z_all[h][:, qt : qt + 1]
            )
            nc.vector.reciprocal(
                r_all[h][:, qt : qt + 1], z_all[h][:, qt : qt + 1]
            )
            nc.vector.tensor_scalar_mul(et[:], et[:], r_all[h][:, qt : qt + 1])
            nc.sync.dma_start(w_out[h, qt * 128 : (qt + 1) * 128, :], et[:])

        def lnz(h):
            # -lnZ: Ln(r) -> transpose -> DRAM roundtrip -> qT_aug row 64
            nl = stat.tile([128, NQT], f32, tag="nl")
            nc.scalar.activation(nl[:], r_all[h][:], LN)
            pst = psum.tile([NQT, 128], f32, tag="big")
            nc.tensor.transpose(pst[:], nl[:], ident[:])
            stg = stat.tile([NQT, 128], f32r, tag="stg")
            nc.vector.tensor_copy(stg[:], pst[:])
            nc.sync.dma_start(
                lnz_dram[h].rearrange("o (a b) -> (o a) b", a=NQT), stg[:]
            )
            nc.sync.dma_start(qT_aug[h][64:65, :], lnz_dram[h][:])

        def s2_tile(h, kt):
            qT = qT_aug[h]
            kT = kT_aug[h]
            ps = psum.tile([128, s], f32, tag="big", name=f"s2_{h}_{kt}")
            lhsT = kT[0:65, kt * 128 : (kt + 1) * 128]
            for qc in range(NCH):
                nc.tensor.matmul(
                    ps[:, qc * 512 : (qc + 1) * 512],
                    lhsT=lhsT,
                    rhs=qT[0:65, qc * 512 : (qc + 1) * 512],
                    start=True,
                    stop=True,
                )
            nc.scalar.activation(et_fulls[h][:, kt, :], ps[:], EXP)

        def pv(h):
            pa = psum.tile([DH, s], f32, tag="big")
            for kt in range(NQT):
                for qc in range(NCH):
                    nc.tensor.matmul(
                        pa[:, qc * 512 : (qc + 1) * 512],
                        lhsT=v_sb[h][:, kt, :],
                        rhs=et_fulls[h][:, kt, qc * 512 : (qc + 1) * 512],
                        start=(kt == 0),
                        stop=(kt == NQT - 1),
                    )
            nc.vector.tensor_copy(attnT[h][:], pa[:])

        for qt in range(NQT):
            s1_tile(0, qt)
        lnz(0)
        for h in range(HPC):
            et_fulls[h] = etpool.tile(
                [128, NQT, s], bf16, tag="eT", name=f"eT{h}"
            )
            if h + 1 < HPC:
                for t in range(NQT):
                    s2_tile(h, t)
                    s1_tile(h + 1, t)
                lnz(h + 1)
            else:
                for t in range(NQT):
                    s2_tile(h, t)
            pv(h)

        # ---- phase O: out[s,:] = sum_h attnT_h.T @ Wo_h^T ----
        for sb in range(NQT):
            po = psum.tile([128, D], f32, tag="big")
            for h in range(HPC):
                lhsT = attnT[h][:, sb * 128 : (sb + 1) * 128]
                nc.tensor.matmul(
                    po[:, 0:512],
                    lhsT=lhsT,
                    rhs=wo_sb[:, h, 0:512],
                    start=(h == 0),
                    stop=(h == HPC - 1),
                )
                nc.tensor.matmul(
                    po[:, 512:768],
                    lhsT=lhsT,
                    rhs=wo_sb[:, h, 512:768],
                    start=(h == 0),
                    stop=(h == HPC - 1),
                )
            ot = opool.tile([128, D], f32, tag="o")
            nc.vector.tensor_copy(ot[:], po[:])
            nc.sync.dma_start(o_out[sb * 128 : (sb + 1) * 128, :], ot[:])

    nc.finalize()
    return nc


def _get_nc():
    if "nc" not in _NC_CACHE:
        _NC_CACHE["nc"] = build_nc()
    return _NC_CACHE["nc"]


def make_in_maps(Q, K, V, Wq, bq, Wk, bk, Wv, bv, Wo, bo, s=S):
    bf = ml_dtypes.bfloat16
    QT = [np.ascontiguousarray(Q[b].T) for b in range(B)]
    KT = [np.ascontiguousarray(K[b].T) for b in range(B)]
    VT = [np.ascontiguousarray(V[b].T).astype(bf) for b in range(B)]
    in_maps = []
    for core in range(NCORES):
        b = core // (NCORES // B)
        h0 = (core % (NCORES // B)) * HPC
        sl = slice(h0 * DH, (h0 + HPC) * DH)
        in_maps.append(
            {
                "xqT": QT[b],
                "xkT": KT[b],
                "xvT": VT[b],
                "wqT": np.ascontiguousarray((Wq[sl, :] * SCALE).T),
                "wkT": np.ascontiguousarray(Wk[sl, :].T),
                "wvT": np.ascontiguousarray(Wv[sl, :].T).astype(bf),
                "woT": np.ascontiguousarray(
                    Wo[:, sl].T.reshape(HPC, DH, D).transpose(1, 0, 2)
                ).astype(bf),
                "bq": np.ascontiguousarray((bq[sl] * SCALE).reshape(HPC, DH).T),
                "bk": np.ascontiguousarray(bk[sl].reshape(HPC, DH).T),
                "ones": np.ones((1, s), np.float32),
            }
        )
    return in_maps


def kernel(**inputs):
    Q = np.asarray(inputs["Q"], np.float32)
    K = np.asarray(inputs["K"], np.float32)
    V = np.asarray(inputs["V"], np.float32)
    Wq = np.asarray(inputs["Wq"], np.float32)
    bq = np.asarray(inputs["bq"], np.float32)
    Wk = np.asarray(inputs["Wk"], np.float32)
    bk = np.asarray(inputs["bk"], np.float32)
    Wv = np.asarray(inputs["Wv"], np.float32)
    bv = np.asarray(inputs["bv"], np.float32)
    Wo = np.asarray(inputs["Wo"], np.float32)
    bo = np.asarray(inputs["bo"], np.float32)

    import os

    from concourse.bass_utils import run_bass_kernel_spmd

    nc = _get_nc()
    in_maps = make_in_maps(Q, K, V, Wq, bq, Wk, bk, Wv, bv, Wo, bo)
    res = run_bass_kernel_spmd(
        nc, in_maps, list(range(NCORES)), trace=bool(os.environ.get("MHA_TRACE"))
    )
    _NC_CACHE["last_res"] = res

    weights = np.empty((B, NH, S, S), np.float32)
    out = np.zeros((B, S, D), np.float32)
    for core in range(NCORES):
        b = core // (NCORES // B)
        h0 = (core % (NCORES // B)) * HPC
        weights[b, h0 : h0 + HPC] = res.results[core]["w_out"]
        out[b] += res.results[core]["o_out"]
    out += (bv @ Wo.T + bo)[None, None, :]
    return (out, weights)
